# revision 2
# baseline (speedup 1.0000x reference)
"""Trainium2 Bass kernel for nn_Blur: 4x4 FIR depthwise blur with pad (2,1).

out[n,c,i,j] = sum_{a,b} K[a,b] * x[n,c, i+1-a, j+1-b]   (zero-padded)

Strategy (8 NeuronCores, pure data parallelism over the 8192 (n,c) slices):
  - fp16 end-to-end on device (host converts): halves HBM traffic vs fp32.
    Quantization error ~4e-4 relative, far under the 2e-2 gate.
  - w-parity interleaved layout: SBUF partition p = 64*(w%2) + h, free dim
    = (slice, w-block jb) with one zero w-block of left/right pad per slice.
    A single rhs column then carries BOTH w-parities of one w-block for all
    64 h rows, so each 128-wide contraction delivers up to 8 useful taps.
  - The 16-tap conv needs only THREE PSUM-accumulated matmuls (block shifts
    delta in {-1,0,+1} along the free dim) instead of four: lhsT_d[(jp_in,u),
    (jp_out,i)] = K[i-u+1, jp_out-jp_in+1-2d] (band in h, parity in w).
  - PE work: 12 x N=512 matmul-columns per 64-slice tile = 41us/core, which
    hides under the fp16 DMA roofline (~48us/core).
  - Host pre-permutes each core's shard into the exact SBUF tile layout so
    every DMA descriptor is one contiguous multi-KB run per partition.
"""

import sys
import types

import numpy as np

import concourse.bacc as bacc
import concourse.mybir as mybir
from concourse.tile import TileContext
from concourse.bass_utils import run_bass_kernel_spmd


def _install_ntff_hook():
    """Best-effort shim: this image's antenv lacks axon_hooks, which the
    trace=True path of run_bass_kernel_spmd imports. Harmless if unused."""
    if "antenv.axon_hooks" in sys.modules:
        return
    try:
        sys.path.insert(0, "/root/.axon_site")
        from trn_agent_boot.trn_boot import _ntff_profile_via_ctypes

        hook = _ntff_profile_via_ctypes("/opt/axon/libaxon_pjrt.so")
        mod = types.ModuleType("antenv.axon_hooks")
        mod.get_axon_ntff_profile_hook = lambda: hook
        mod.set_axon_ntff_profile_hook = lambda h: None
        sys.modules["antenv.axon_hooks"] = mod
    except Exception:
        pass


_install_ntff_hook()

N_CORES = 8
B, C, H, W = 32, 256, 64, 64
NSLICES = B * C                      # 8192
SLICES_PER_CORE = NSLICES // N_CORES  # 1024
TILE_SLICES = 64                     # slices per SBUF tile
JB = W // 2                          # 32 w-blocks of 2 per slice
JBP = JB + 2                         # +1 zero block left, +1 right
GQ = 16                              # slices per PSUM group (N = 16*32 = 512)
F16 = mybir.dt.float16
F32 = mybir.dt.float32

_NC_CACHE = {}


def _build_wmat(K: np.ndarray) -> np.ndarray:
    """(3, 128, 128) fp16 lhsT matrices for block shifts d = 0, -1, +1."""
    K = np.asarray(K, np.float32)
    wmat = np.zeros((3, 128, 128), np.float32)
    for di, d in enumerate((0, -1, 1)):
        L = wmat[di]
        for jpi in range(2):
            for jpo in range(2):
                b = jpo - jpi + 1 - 2 * d
                if not (0 <= b < 4):
                    continue
                for i in range(H):
                    for a in range(4):
                        u = i + 1 - a
                        if 0 <= u < H:
                            L[64 * jpi + u, 64 * jpo + i] += K[a, b]
    return wmat.astype(np.float16)


WARMUP_MMS = 12


def _build_nc(slices_per_core: int = SLICES_PER_CORE):
    ntiles = slices_per_core // TILE_SLICES
    nc = bacc.Bacc("TRN2", target_bir_lowering=False, debug=False)
    # DRAM layouts are the SBUF tile layouts (host pre-/post-permutes):
    #   x: [tile, p=(jp h), (s jbp)]  with jbp = [pad, 32 data blocks, pad]
    x = nc.dram_tensor(
        "x", [ntiles, 128, TILE_SLICES * JBP], F16, kind="ExternalInput"
    ).ap()
    wm = nc.dram_tensor("w", [3, 128, 128], F16, kind="ExternalInput").ap()
    y = nc.dram_tensor(
        "y", [ntiles, 128, TILE_SLICES * JB], F16, kind="ExternalOutput"
    ).ap()
    # sink for the PE warm-up matmuls (kept alive so DCE can't drop them)
    warm_out = nc.dram_tensor("warm", [128, 4], F32, kind="ExternalOutput").ap()

    NG = TILE_SLICES // GQ  # PSUM groups per tile = 4

    with TileContext(nc) as tc:
        with (
            tc.tile_pool(name="wpool", bufs=1) as wpool,
            tc.tile_pool(name="xpool", bufs=6) as xpool,
            tc.tile_pool(name="opool", bufs=6) as opool,
            tc.tile_pool(name="pspool", bufs=8, space="PSUM") as pspool,
        ):
            wsb = wpool.tile([128, 3, 128], F16, name="wsb")
            nc.sync.dma_start(wsb[:], wm.rearrange("d k m -> k d m"))

            # PE warm-up: ~4us of matmuls on the weight tile while the first
            # input tiles stream in, so the HAM clock gate opens (1.2 ->
            # 2.4 GHz) before the real matmuls start. Only depends on wsb.
            wscratch = wpool.tile([128, 4], F32, name="wscratch")
            if WARMUP_MMS:
                wps = pspool.tile([128, 384], F32, name="wps", tag="ps")
                for r in range(WARMUP_MMS):
                    nc.tensor.matmul(
                        wps[:],
                        wsb[:, 0, :],
                        wsb.rearrange("p d m -> p (d m)"),
                        start=(r == 0),
                        stop=(r == WARMUP_MMS - 1),
                    )
                nc.vector.tensor_copy(wscratch[:], wps[:, 0:4])
            else:
                nc.vector.tensor_copy(wscratch[:], wsb[:, 0, 0:4].bitcast(F32))
            nc.sync.dma_start(warm_out, wscratch[:])

            for t in range(ntiles):
                xt = xpool.tile([128, TILE_SLICES, JBP], F16, name="xt")
                nc.sync.dma_start(xt[:], x[t])

                ot = opool.tile([128, TILE_SLICES, JB], F16, name="ot")
                pss = [
                    pspool.tile([128, GQ * JB], F32, name="ps") for _ in range(NG)
                ]
                # d-outer loop: 3 stationary loads per tile, each streaming
                # 4 x 512 columns before the next LDWEIGHTS.
                for di, d in enumerate((0, -1, 1)):
                    for q in range(NG):
                        nc.tensor.matmul(
                            pss[q][:],
                            wsb[:, di, :],
                            xt[:, GQ * q : GQ * (q + 1), 1 + d : 1 + d + JB],
                            start=(di == 0),
                            stop=(di == 2),
                        )
                for q in range(NG):
                    # alternate copy engine: DVE and ACT share the load
                    if q % 2 == 0:
                        nc.vector.tensor_copy(
                            ot[:, GQ * q : GQ * (q + 1), :], pss[q][:]
                        )
                    else:
                        nc.scalar.copy(
                            ot[:, GQ * q : GQ * (q + 1), :], pss[q][:]
                        )
                # one store per tile (512 KB, 4 KB/partition contiguous).
                # Alternate rings so the store stream never head-of-line
                # blocks the SP ring feeding loads.
                store_eng = nc.scalar if t % 2 == 0 else nc.sync
                store_eng.dma_start(y[t], ot[:])

    nc.compile()
    return nc


def get_nc(slices_per_core: int = SLICES_PER_CORE):
    if slices_per_core not in _NC_CACHE:
        _NC_CACHE[slices_per_core] = _build_nc(slices_per_core)
    return _NC_CACHE[slices_per_core]


def _pack_input(xs: np.ndarray) -> np.ndarray:
    """[S, H, W] fp16 -> [S/64, 128, 64*JBP] in the SBUF tile layout."""
    s = xs.shape[0]
    ntiles = s // TILE_SLICES
    # [s, jp, h, jbp] with jbp zero-padded on both block ends
    v = np.zeros((s, 2, H, JBP), np.float16)
    v[:, 0, :, 1 : 1 + JB] = xs[:, :, 0::2]
    v[:, 1, :, 1 : 1 + JB] = xs[:, :, 1::2]
    # (t, s, jp, h, jbp) -> (t, jp, h, s, jbp)
    v = v.reshape(ntiles, TILE_SLICES, 2, H, JBP).transpose(0, 2, 3, 1, 4)
    return np.ascontiguousarray(v.reshape(ntiles, 128, TILE_SLICES * JBP))


def _unpack_output(yp: np.ndarray) -> np.ndarray:
    """[S/64, 128, 64*JB] fp16 -> [S, H, W] fp16."""
    ntiles = yp.shape[0]
    v = yp.reshape(ntiles, 2, H, TILE_SLICES, JB)        # [t, jp, i, s, jb]
    out = np.empty((ntiles, TILE_SLICES, H, W), np.float16)
    out[:, :, :, 0::2] = v[:, 0].transpose(0, 2, 1, 3)
    out[:, :, :, 1::2] = v[:, 1].transpose(0, 2, 1, 3)
    return out.reshape(ntiles * TILE_SLICES, H, W)


def kernel(x: np.ndarray, kernel: np.ndarray, _trace: bool = False, **_tkw):
    xh = np.asarray(x).astype(np.float16)
    wmat = _build_wmat(kernel)
    b, c, h, w = x.shape
    xs = xh.reshape(b * c, h, w)
    spc = (b * c) // N_CORES
    nc = get_nc(spc)
    in_maps = [
        {"x": _pack_input(xs[k * spc : (k + 1) * spc]), "w": wmat}
        for k in range(N_CORES)
    ]
    res = run_bass_kernel_spmd(
        nc, in_maps, list(range(N_CORES)), trace=_trace, **_tkw
    )
    out = np.concatenate(
        [_unpack_output(res.results[k]["y"]) for k in range(N_CORES)], axis=0
    )
    result = out.reshape(b, c, h, w).astype(np.float32)
    if _trace:
        return result, res
    return result
